# revision 19
# baseline (speedup 1.0000x reference)
"""Deep MLP without skip connections, data-parallel over 8 NeuronCores.

Network: sigmoid(x @ W_in + b_in) -> 98x [sigmoid(h @ W_l + b_l)] -> h @ W_out + b_out
  x: [524288, 10], hidden 64, out 2.

Strategy (per core, batch shard of 65536):
  - Activations live as [feature, batch] (features on partitions). Two
    independent 2048-col batch "lane groups" stacked on partitions 0-63 /
    64-127 so ScalarE (sigmoid, the bottleneck engine at ~2.7ms/core floor)
    runs at full 128-lane rate.
  - Weights are host-prepped BLOCK-DIAGONAL [128,128]: one full-array matmul
    per 512-col slice computes both lane groups at once (1 col/cycle).
  - Per hidden layer per supertile: 4 matmuls into a [128, 2048] PSUM tile
    (4 banks), then ONE ScalarE instruction does bias + sigmoid -> SBUF.
  - Two supertiles (4096 batch) processed as an interleaved pair so PE of one
    overlaps ACT of the other. PSUM: 2 tiles x 4 banks = all 8 banks.
  - Layer 0: group A's x^T on partitions 0-9, group B's on 10-19, with a
    [20,128] block W_in -> same single-matmul form.
  - x loaded transposed by strided DMA; y stored transposed likewise.
  - Matmul dtype float32r (tf32-like, 1 col/cycle at free dim >= 256):
    measured best both in precision (1.5e-3 rel) and speed vs bf16/fp16.
"""

import os
import numpy as np

N_CORES = 8
B_TOTAL = 524288
B_LOCAL = B_TOTAL // N_CORES  # 65536
D_IN = 10
H = 64
N_HID = 98
D_OUT = 2

COLS = 2048                # batch columns per lane group per supertile
NSLICE = 512               # matmul free dim = one PSUM bank (fp32)
NSL = COLS // NSLICE       # 4 matmul slices
SUPER = 2 * COLS           # 4096 batch rows per supertile
N_SUPER = B_LOCAL // SUPER # 16

MM_DTYPE = os.environ.get("KERNEL_MM_DTYPE", "f32r")  # "f32r" | "bf16" | "fp16"

_CACHE = {}


def _build(n_super=N_SUPER, repeat=1, act_cols=COLS, mm_dtype=None):
    import concourse.mybir as mybir
    import concourse.tile as tile
    from concourse import bacc
    from contextlib import ExitStack

    mm_dtype = mm_dtype or MM_DTYPE
    f32 = mybir.dt.float32
    wdt = {"bf16": mybir.dt.bfloat16, "f32r": mybir.dt.float32r,
           "fp16": mybir.dt.float16}[mm_dtype]
    SIG = mybir.ActivationFunctionType.Sigmoid

    nc = bacc.Bacc("TRN2", target_bir_lowering=False, debug=False,
                   num_devices=N_CORES)

    x_d = nc.dram_tensor("x", [B_LOCAL, D_IN], wdt, kind="ExternalInput").ap()
    wib_d = nc.dram_tensor("w_in_blk", [2 * D_IN, 128], wdt,
                           kind="ExternalInput").ap()
    whb_d = nc.dram_tensor("w_hid_blk", [N_HID, 128, 128], wdt,
                           kind="ExternalInput").ap()
    wob_d = nc.dram_tensor("w_out_blk", [128, 128], wdt,
                           kind="ExternalInput").ap()
    bi_d = nc.dram_tensor("bias_in", [128, 1], f32, kind="ExternalInput").ap()
    bh_d = nc.dram_tensor("bias_hid", [128, N_HID], f32,
                          kind="ExternalInput").ap()
    bo_d = nc.dram_tensor("bias_out", [128, 1], f32, kind="ExternalInput").ap()
    y_d = nc.dram_tensor("y", [B_LOCAL, D_OUT], f32, kind="ExternalOutput").ap()

    with tile.TileContext(nc) as tc:
        with ExitStack() as ctx:
            singles = ctx.enter_context(tc.tile_pool(name="singles", bufs=1))
            x_pool = ctx.enter_context(tc.tile_pool(name="xp", bufs=4))
            h_pool = ctx.enter_context(tc.tile_pool(name="hp", bufs=8))
            y_pool = ctx.enter_context(tc.tile_pool(name="yp", bufs=4))
            psum_pool = ctx.enter_context(
                tc.tile_pool(name="pp", bufs=2, space="PSUM"))

            w_hid_sb = singles.tile([128, N_HID, 128], wdt)
            w_in_sb = singles.tile([2 * D_IN, 128], wdt)
            w_out_sb = singles.tile([128, 128], wdt)
            b_hid_sb = singles.tile([128, N_HID], f32)
            b_in_sb = singles.tile([128, 1], f32)
            b_out_sb = singles.tile([128, 1], f32)
            nc.sync.dma_start(out=w_hid_sb,
                              in_=whb_d.rearrange("l k m -> k l m"))
            nc.sync.dma_start(out=w_in_sb, in_=wib_d)
            nc.sync.dma_start(out=w_out_sb, in_=wob_d)
            nc.sync.dma_start(out=b_hid_sb, in_=bh_d)
            nc.sync.dma_start(out=b_in_sb, in_=bi_d)
            nc.sync.dma_start(out=b_out_sb, in_=bo_d)

            x_t = x_d.rearrange("b k -> k b")      # [10, B_LOCAL] strided view
            y_t = y_d.rearrange("b o -> o b")      # [2, B_LOCAL] strided view

            def slices():
                return [slice(n * NSLICE, (n + 1) * NSLICE) for n in range(NSL)]

            for pair_rep in range(repeat * (n_super // 2)):
                pair = pair_rep % (n_super // 2)
                pst = (2 * pair, 2 * pair + 1)
                # x^T loads: lane group A -> partitions 0-9, B -> 10-19
                xt = {}
                for s in pst:
                    xt[s] = x_pool.tile([2 * D_IN, COLS], wdt, tag="xt",
                                        name="xt")
                    for p0, c0 in ((0, s * SUPER), (D_IN, s * SUPER + COLS)):
                        nc.sync.dma_start(out=xt[s][p0:p0 + D_IN, :],
                                          in_=x_t[:, c0:c0 + COLS])
                # layer 0
                h = {}
                for s in pst:
                    ps = psum_pool.tile([128, COLS], f32, tag="ps", name="ps")
                    for cs in slices():
                        nc.tensor.matmul(out=ps[:, cs], lhsT=w_in_sb,
                                         rhs=xt[s][:, cs],
                                         start=True, stop=True)
                    hn = h_pool.tile([128, COLS], wdt, tag="h", name="h")
                    nc.scalar.activation(hn, ps, SIG, bias=b_in_sb[:, 0:1])
                    h[s] = hn
                # 98 hidden layers, pair-interleaved
                for l in range(N_HID):
                    for s in pst:
                        ps = psum_pool.tile([128, COLS], f32, tag="ps",
                                            name="ps")
                        for cs in slices():
                            nc.tensor.matmul(out=ps[:, cs],
                                             lhsT=w_hid_sb[:, l, :],
                                             rhs=h[s][:, cs],
                                             start=True, stop=True)
                        hn = h_pool.tile([128, COLS], wdt, tag="h", name="h")
                        nc.scalar.activation(hn[:, :act_cols],
                                             ps[:, :act_cols], SIG,
                                             bias=b_hid_sb[:, l:l + 1])
                        h[s] = hn
                # output layer (no sigmoid); only partitions 0,1,64,65 useful
                for s in pst:
                    ps = psum_pool.tile([128, COLS], f32, tag="ps", name="ps")
                    for cs in slices():
                        nc.tensor.matmul(out=ps[:, cs], lhsT=w_out_sb,
                                         rhs=h[s][:, cs],
                                         start=True, stop=True)
                    yt = y_pool.tile([128, COLS], f32, tag="yt", name="yt")
                    # bias-add on DVE (idle engine) to keep ScalarE on sigmoid
                    for p0 in (0, 64):
                        nc.vector.tensor_scalar_add(
                            yt[p0:p0 + D_OUT, :], ps[p0:p0 + D_OUT, :],
                            b_out_sb[p0:p0 + D_OUT, 0:1])
                    for p0, c0 in ((0, s * SUPER), (64, s * SUPER + COLS)):
                        nc.sync.dma_start(out=y_t[:, c0:c0 + COLS],
                                          in_=yt[p0:p0 + D_OUT, :])

    nc.compile()
    return nc


def get_nc():
    if "nc" not in _CACHE:
        _CACHE["nc"] = _build()
    return _CACHE["nc"]


def _prep_host_inputs(x, W_in, b_in, W_hid, b_hid, W_out, b_out,
                      mm_dtype=None):
    mm_dtype = mm_dtype or MM_DTYPE
    f = np.float32
    if mm_dtype == "bf16":
        import ml_dtypes
        wnp = ml_dtypes.bfloat16
    elif mm_dtype == "fp16":
        wnp = np.float16
    else:
        wnp = np.float32
    x = np.ascontiguousarray(np.asarray(x, dtype=f), dtype=wnp)
    W_in = np.asarray(W_in, dtype=f)
    b_in = np.asarray(b_in, dtype=f)
    W_hid = np.asarray(W_hid, dtype=f)
    b_hid = np.asarray(b_hid, dtype=f)
    W_out = np.asarray(W_out, dtype=f)
    b_out = np.asarray(b_out, dtype=f)

    wib = np.zeros((2 * D_IN, 128), f)
    wib[:D_IN, :H] = W_in
    wib[D_IN:, H:] = W_in

    whb = np.zeros((N_HID, 128, 128), f)
    whb[:, :H, :H] = W_hid
    whb[:, H:, H:] = W_hid

    wob = np.zeros((128, 128), f)
    wob[:H, :D_OUT] = W_out
    wob[H:, H + 0:H + D_OUT] = W_out

    bi = np.concatenate([b_in, b_in]).reshape(128, 1).astype(f)
    bh = np.concatenate([b_hid.T, b_hid.T], axis=0).astype(f)  # [128, 98]
    bo = np.zeros((128, 1), f)
    bo[0:D_OUT, 0] = b_out
    bo[H:H + D_OUT, 0] = b_out

    shared = {
        "w_in_blk": wib.astype(wnp), "w_hid_blk": whb.astype(wnp),
        "w_out_blk": wob.astype(wnp),
        "bias_in": bi, "bias_hid": bh, "bias_out": bo,
    }
    return x, shared


def kernel(x, W_in, b_in, W_hid, b_hid, W_out, b_out, _want_results=False):
    trace = bool(int(os.environ.get("KERNEL_TRACE", "0")))
    if not trace:
        # NTFF tracing needs antenv.axon_hooks (absent in this container);
        # make sure a stray BASS_TRACE in the environment can't crash us.
        os.environ["BASS_NEVER_TRACE"] = "1"
    from concourse.bass_utils import run_bass_kernel_spmd

    nc = get_nc()
    x, shared = _prep_host_inputs(x, W_in, b_in, W_hid, b_hid, W_out, b_out)
    in_maps = [
        {"x": x[i * B_LOCAL:(i + 1) * B_LOCAL], **shared}
        for i in range(N_CORES)
    ]
    res = run_bass_kernel_spmd(
        nc, in_maps, list(range(N_CORES)),
        trace=trace,
    )
    y = np.concatenate([r["y"] for r in res.results], axis=0)
    if _want_results:
        return y, res
    return y
